# revision 1
# baseline (speedup 1.0000x reference)
"""Trainium2 Bass kernel for 2D attention with relative-position augmentation.

Problem shapes (hardcoded): inputs [8, 32, 32, 768] fp32 (q|k|v packed on the
channel axis, 256 each), key_rel_w/key_rel_h [63, 32] fp32.
Output: [8, 32, 32, 256] fp32.

Sharding: data-parallel over batch - core b gets batch b (8 cores, no
collectives needed).

Per-core math (N = 32*32 = 1024 tokens, 8 heads, head dim 32):
  L[n, m] = Q[n].K[m] + qdw[n, y2(m)-y(n)+31] + qdh[n, x2(m)-x(n)+31]
  out[n]  = softmax_m(L[n, :] / sqrt(32)) @ V
where qdw = Q @ key_rel_w^T, qdh = Q @ key_rel_h^T and n=(x,y), m=(x2,y2).

Kernel formulation:
  * We compute L^T (m on partitions, n on free dim) so that the attention
    matmul can consume P^T = exp(L^T) directly as the stationary operand.
  * The two relative-logit terms are folded into the SAME matmul as Q.K by
    extending the contraction dim from 32 to 96:
       lhsT rows  0-31: K^T            rhs rows  0-31: Q^T
       lhsT rows 32-63: Aw[y',m]=[y2(m)==y']   rhs rows 32-63: Bw[y',n]=qdw^T[y'-y(n)+31, n]
       lhsT rows 64-95: Ah[x',m]=[x2(m)==x']   rhs rows 64-95: Bh[x',n]=qdh^T[x'-x(n)+31, n]
    The Bw/Bh rows are partition-shifted copies of qdw^T/qdh^T (one copy per
    y / x value, batched over heads), since the shift only depends on
    y(n) / x(n).
  * Softmax skips the max-subtraction (logits are ~N(0,1), max << 80), and
    the 1/sqrt(32) scale is folded into the Exp activation's free pre-scale.
  * The row sums are fused into the attention matmul by appending a ones
    column to V: [A | s] = P @ [V | 1]; normalization is then a per-partition
    reciprocal+multiply.
"""

import numpy as np

import concourse.bacc as bacc
import concourse.mybir as mybir
from concourse.tile import TileContext
from concourse.bass_utils import run_bass_kernel_spmd

F32 = mybir.dt.float32
BF16 = mybir.dt.bfloat16
AF = mybir.ActivationFunctionType
ALU = mybir.AluOpType

N_CORES = 8
N = 1024          # tokens per batch (32 x 32)
NH = 8            # heads
DK = 32           # head dim
EXP_SCALE = float(1.0 / np.sqrt(32.0))

_CACHE = {}


def _emit(tc, x, rw, rh, out):
    nc = tc.nc

    with tc.tile_pool(name="big", bufs=1) as big, \
         tc.tile_pool(name="dram", bufs=1, space="DRAM") as dram:

        # ---- Q, K: fp32 HBM -> bf16 HBM (SWDGE cast) -> xbar transpose-load
        # to [channel, token] layout.
        qbf_d = dram.tile([N, 256], BF16, name="qbf_d")
        kbf_d = dram.tile([N, 256], BF16, name="kbf_d")
        nc.gpsimd.dma_start(out=qbf_d[:], in_=x[:, 0:256])
        nc.gpsimd.dma_start(out=kbf_d[:], in_=x[:, 256:512])

        qt0 = big.tile([128, N], BF16, name="qt0")
        qt1 = big.tile([128, N], BF16, name="qt1")
        kt0 = big.tile([128, N], BF16, name="kt0")
        kt1 = big.tile([128, N], BF16, name="kt1")
        nc.sync.dma_start(out=qt0[:], in_=qbf_d[:, 0:128], transpose=True)
        nc.sync.dma_start(out=qt1[:], in_=qbf_d[:, 128:256], transpose=True)
        nc.sync.dma_start(out=kt0[:], in_=kbf_d[:, 0:128], transpose=True)
        nc.sync.dma_start(out=kt1[:], in_=kbf_d[:, 128:256], transpose=True)

        # ---- V natural layout + ones column -> Vp [128, (mchunk, head, 33)]
        xv = big.tile([128, 8 * 256], F32, name="xv")
        nc.sync.dma_start(
            out=xv[:].rearrange("p (t c) -> p t c", c=256),
            in_=x.rearrange("(t p) c -> p t c", p=128)[:, :, 512:768],
        )
        vp = big.tile([128, 8 * NH * 33], BF16, name="vp")
        vp_r = vp[:].rearrange("p (t h c) -> p t h c", t=8, h=NH)
        xv_r = xv[:].rearrange("p (t h c) -> p t h c", t=8, h=NH)
        nc.vector.tensor_copy(vp_r[:, :, :, 0:32], xv_r)
        nc.vector.memset(vp_r[:, :, :, 32:33], 1.0)

        # ---- rel tables -> RT [32, 128] bf16  (cols: 0-63 w-table^T, 64-127
        # h-table^T; cols 63 and 127 are zero padding)
        rel4 = big.tile([32, 128], F32, name="rel4")
        nc.vector.memset(rel4[:, :], 0.0)
        nc.sync.dma_start(out=rel4[0:32, 0:32], in_=rw[0:32, :])
        nc.sync.dma_start(out=rel4[0:31, 32:64], in_=rw[32:63, :])
        nc.sync.dma_start(out=rel4[0:32, 64:96], in_=rh[0:32, :])
        nc.sync.dma_start(out=rel4[0:31, 96:128], in_=rh[32:63, :])
        rtf = big.tile([32, 128], F32, name="rtf")
        nc.vector.transpose(rtf[:, :], rel4[:, :])  # 4x 32x32 block transpose
        rt = big.tile([32, 128], BF16, name="rt")
        nc.vector.tensor_copy(rt[:], rtf[:])

        # ---- one-hot selector rows OH [64, 1024] bf16
        #   rows  0-31: Aw[y', m] = 1 iff m % 32 == y'
        #   rows 32-63: Ah[x', m] = 1 iff m // 32 == x'
        oh = big.tile([64, N], BF16, name="oh")
        itw = big.tile([32, N], mybir.dt.int32, name="itw")
        ith = big.tile([32, N], mybir.dt.int32, name="ith")
        nc.gpsimd.iota(
            itw[:].rearrange("p (mx my) -> p mx my", mx=32),
            pattern=[[0, 32], [1, 32]], base=0, channel_multiplier=-1,
        )
        nc.gpsimd.iota(
            ith[:].rearrange("p (mx my) -> p mx my", mx=32),
            pattern=[[1, 32], [0, 32]], base=0, channel_multiplier=-1,
        )
        nc.vector.tensor_scalar(oh[0:32, :], itw[:], 0, None, ALU.is_equal)
        nc.vector.tensor_scalar(oh[32:64, :], ith[:], 0, None, ALU.is_equal)

        # ---- extended stationary / moving operands, one 1024-col block per
        # head: KE = [K^T; Aw; Ah], QE = [Q^T; Bw; Bh]
        ke = big.tile([96, NH * N], BF16, name="ke")
        qe = big.tile([96, NH * N], BF16, name="qe")
        for h in range(NH):
            qt = qt0 if h < 4 else qt1
            kt = kt0 if h < 4 else kt1
            p0 = (h % 4) * 32
            nc.vector.tensor_copy(qe[0:32, h * N:(h + 1) * N], qt[p0:p0 + 32, :])
            nc.vector.tensor_copy(ke[0:32, h * N:(h + 1) * N], kt[p0:p0 + 32, :])
            nc.vector.tensor_copy(ke[32:64, h * N:(h + 1) * N], oh[0:32, :])
            nc.vector.tensor_copy(ke[64:96, h * N:(h + 1) * N], oh[32:64, :])

        # ---- Bw/Bh rows of QE, computed directly with shifted free-slices of
        # RT as the stationary operand (partition starts must be 32-aligned,
        # so the shift must live on the free axis):
        #   Bw[y', n]|y(n)=y = rel_w[31-y+y'] . Q[n] -> lhsT = rt[:, 31-y : 63-y]
        #   Bh[x', n]|x(n)=x = rel_h[31-x+x'] . Q[n] -> lhsT = rt[:, 95-x : 127-x]
        # Two heads per PSUM tile; w rows at partitions 0-31 (free (y,hh,x)),
        # h rows at partitions 32-63 (free (xx,hh,y)).
        qe_v = qe[:].rearrange("p (h nx ny) -> p h nx ny", h=NH, nx=32)
        with tc.tile_pool(name="bpp", bufs=2, space="PSUM") as bpp:
            for g in range(4):
                b_ps = bpp.tile([64, 2048], F32, name="b_ps")
                for y in range(32):
                    nc.tensor.matmul(
                        b_ps[0:32, y * 64:(y + 1) * 64],
                        rt[:, 31 - y:63 - y],
                        qe_v[0:32, 2 * g:2 * g + 2, :, y:y + 1],
                        start=True, stop=True,
                    )
                    nc.tensor.matmul(
                        b_ps[32:64, y * 64:(y + 1) * 64],
                        rt[:, 95 - y:127 - y],
                        qe_v[0:32, 2 * g:2 * g + 2, y:y + 1, :],
                        start=True, stop=True,
                    )
                bw = b_ps[0:32, :].rearrange("p (y hh x) -> p hh x y", y=32, hh=2)
                bh = b_ps[32:64, :].rearrange("p (x hh y) -> p hh x y", x=32, hh=2)
                nc.vector.tensor_copy(qe_v[32:64, 2 * g:2 * g + 2, :, :], bw)
                nc.vector.tensor_copy(qe_v[64:96, 2 * g:2 * g + 2, :, :], bh)

        # ---- main loop: per (head, m-chunk): L^T matmul (K=96), exp, AV
        out_r = out.rearrange("(j p) c -> p j c", p=128)
        with tc.tile_pool(name="lpp", bufs=2, space="PSUM") as lpp, \
             tc.tile_pool(name="app", bufs=3, space="PSUM") as app, \
             tc.tile_pool(name="ptp", bufs=10) as ptp, \
             tc.tile_pool(name="outp", bufs=4) as outp:
            for h in range(NH):
                a_ps = app.tile([128, 288], F32, name="a_ps")
                pts = []
                for i in range(8):
                    l_ps = lpp.tile([128, N], F32, name="l_ps")
                    for c in range(2):
                        nc.tensor.matmul(
                            l_ps[:, c * 512:(c + 1) * 512],
                            ke[:, h * N + i * 128: h * N + i * 128 + 128],
                            qe[:, h * N + c * 512: h * N + (c + 1) * 512],
                            start=True, stop=True,
                        )
                    pt = ptp.tile([128, N], BF16, name="pt")
                    nc.scalar.activation(pt[:], l_ps[:], AF.Exp, scale=EXP_SCALE)
                    pts.append(pt)
                # One accumulation group per a_ps slice at a time (PSUM
                # accumulation groups are per-bank), hence j outer.
                for j in range(8):
                    for i in range(8):
                        nc.tensor.matmul(
                            a_ps[:, j * 36: j * 36 + 33],
                            pts[i][:, j * 128:(j + 1) * 128],
                            vp[:, (i * NH + h) * 33: (i * NH + h) * 33 + 33],
                            start=(i == 0), stop=(i == 7),
                        )
                # normalize: out[n, c] = A[n, c] / s[n], then store
                r = outp.tile([128, 8], F32, name="r")
                a_r = a_ps[:].rearrange("p (j c) -> p j c", c=36)
                r_r = r[:].rearrange("p (j o) -> p j o", o=1)
                nc.vector.reciprocal(r_r, a_r[:, :, 32:33])
                o_sb = outp.tile([128, 256], F32, name="o_sb")
                o_r = o_sb[:].rearrange("p (j c) -> p j c", c=32)
                for j in range(8):
                    nc.vector.tensor_scalar_mul(
                        o_r[:, j:j + 1, :], a_r[:, j:j + 1, 0:32], r_r[:, j:j + 1, :]
                    )
                nc.sync.dma_start(
                    out=out_r[:, :, h * 32:(h + 1) * 32], in_=o_r[:, :, :]
                )


def build_nc():
    if "nc" in _CACHE:
        return _CACHE["nc"]
    nc = bacc.Bacc(
        "TRN2", target_bir_lowering=False, debug=False, num_devices=N_CORES
    )
    x = nc.dram_tensor("x", [N, 768], F32, kind="ExternalInput")
    rw = nc.dram_tensor("rw", [63, 32], F32, kind="ExternalInput")
    rh = nc.dram_tensor("rh", [63, 32], F32, kind="ExternalInput")
    out = nc.dram_tensor("out", [N, 256], F32, kind="ExternalOutput")
    with TileContext(nc) as tc:
        _emit(tc, x.ap(), rw.ap(), rh.ap(), out.ap())
    nc.compile()
    _CACHE["nc"] = nc
    return nc


def kernel(inputs, key_rel_w, key_rel_h):
    B = inputs.shape[0]
    assert inputs.shape == (8, 32, 32, 768), inputs.shape
    nc = build_nc()
    x_full = np.ascontiguousarray(inputs.reshape(B, N, 768), dtype=np.float32)
    rw = np.ascontiguousarray(key_rel_w, dtype=np.float32)
    rh = np.ascontiguousarray(key_rel_h, dtype=np.float32)
    in_maps = [{"x": x_full[b], "rw": rw, "rh": rh} for b in range(N_CORES)]
    res = run_bass_kernel_spmd(nc, in_maps, list(range(N_CORES)))
    return np.stack(
        [res.results[b]["out"].reshape(32, 32, 256) for b in range(N_CORES)]
    )


if __name__ == "__main__":
    rng = np.random.default_rng(0)
    inputs = rng.standard_normal((8, 32, 32, 768), dtype=np.float32)
    rw = rng.standard_normal((63, 32), dtype=np.float32) * 32 ** -0.5
    rh = rng.standard_normal((63, 32), dtype=np.float32) * 32 ** -0.5
    o = kernel(inputs, rw, rh)
    print(o.shape, o.dtype, float(np.abs(o).max()))

